# revision 30
# baseline (speedup 1.0000x reference)
"""Trainium2 Bass kernel for nn_AdaptiveActivationBlock (grouped deformable
conv block: offset conv -> affine-grid bilinear deform conv -> BN -> residual
ReLU).

Strategy (8 NeuronCores, SPMD, zero collectives):
  - The affine-grid matrix `reg` is folded into the offset-conv weights on the
    host, so the PE produces per-tap offsets (offy_k, offx_k) directly.
  - Bilinear sampling weights are tent functions tent(t)=relu(1-|t|) of the
    offsets: branchless 3x3 window per tap (offsets clamped to +-0.999).
  - The channel contraction commutes with bilinear sampling:
    F_k = W_k @ x (1x1 conv per tap, PE), then
    out = sum_{k,dy,dx} u[k,dy,dx] * shift(F_k)  (81 masked-FMA passes, DVE),
    with u broadcast across the 32 output-channel partitions via DMA.
  - u-broadcast routes through DRAM in bf16 with stride-4 interleaved dest
    partitions (o-major row layout p = o*4+b) so each sub-DMA spreads across
    all 16 SDMA engines (~340 GB/s vs ~100 GB/s naive SBUF broadcast).
  - Sharding: (group, H-octant) blocks of [128 rows, 12x96 px];
    136 blocks / 8 cores = 17 each, perfectly balanced.
"""
import numpy as np
from ml_dtypes import bfloat16

import concourse.bass as bass
import concourse.tile as tile
from concourse.tile_rust import add_dep_helper
from concourse import bacc, mybir
from concourse.bass_utils import run_bass_kernel_spmd

G = 17
B = 4
CG = 32
H = W = 96
BN_EPS = 1e-5
CLAMP = 0.999
NB = 17            # blocks per core
ROWS = 12          # output rows per block
GR, GC = 16, 100   # padded slab grid
NGRID = GR * GC
NOUT = ROWS * W
KY = [-1, -1, -1, 0, 0, 0, 1, 1, 1]
KX = [-1, 0, 1, -1, 0, 1, -1, 0, 1]
F32 = mybir.dt.float32
BF16 = mybir.dt.bfloat16
ALU = mybir.AluOpType

BLOCKS = [(g, o) for g in range(G) for o in range(8)]   # 136 = 8*17

_nc_cache = None


def _prep_weights(w_off, b_off, w_def, gamma, beta, run_mean, run_var):
    ky = np.array(KY, np.float32)
    kx = np.array(KX, np.float32)
    w_off = w_off.reshape(G, 2, 3, CG, 3, 3)
    b_off = b_off.reshape(G, 2, 3)
    wofk = (ky[None, None, :, None, None, None] * w_off[:, :, 0][:, :, None]
            + kx[None, None, :, None, None, None] * w_off[:, :, 1][:, :, None]
            + w_off[:, :, 2][:, :, None])               # [G,2,9,CG,3,3]
    bofk = (ky[None, None, :] * b_off[:, :, 0:1]
            + kx[None, None, :] * b_off[:, :, 1:2]
            + b_off[:, :, 2:3])                          # [G,2,9]
    s = gamma / np.sqrt(run_var + BN_EPS)
    t = beta - run_mean * s
    wdef = w_def.reshape(G, CG, CG, 3, 3) * s.reshape(G, CG, 1, 1, 1)
    wdef = wdef.reshape(G, CG, CG, 9)                    # [G,o,i,k]
    return wofk, bofk, wdef, t.reshape(G, CG)


def _host_pack(x, wofk, bofk, wdef, tbias):
    """Build the 8 per-core input dicts.

    Row layouts: contraction rows (xslab) are b-major (p = b*32+i);
    output rows (F/acc/residual/out) are o-major (p = o*4+b)."""
    maps = []
    for c in range(8):
        blks = BLOCKS[c * NB:(c + 1) * NB]
        xslab = np.zeros((NB, 128, NGRID), np.float32)
        xres = np.zeros((NB, 128, NOUT), np.float32)
        wdefk = np.zeros((NB, 128, 9, 128), np.float32)
        woffm = np.zeros((NB, 128, 9, 72), np.float32)
        boffb = np.zeros((NB, 2, 36, 1), np.float32)
        tbv = np.zeros((NB, 128, 1), np.float32)
        for j, (g, oc) in enumerate(blks):
            r0 = 12 * oc
            xg = x[:, g * CG:(g + 1) * CG]              # [B,32,96,96]
            slab = np.zeros((B, CG, GR, GC), np.float32)
            rlo, rhi = max(r0 - 2, 0), min(r0 + 14, H)
            slab[:, :, rlo - (r0 - 2):rlo - (r0 - 2) + (rhi - rlo), 2:98] = \
                xg[:, :, rlo:rhi]
            xslab[j] = slab.reshape(B * CG, NGRID)
            # residual in o-major rows: row o*4+b = x[b, o-channel]
            xres[j] = xg[:, :, r0:r0 + 12].transpose(1, 0, 2, 3).reshape(128, NOUT)
            for b in range(B):
                sl = slice(b * CG, (b + 1) * CG)
                # F matmul lhsT[(b,i), k, o*4+b] = wdef[g][o,i,k]
                wdefk[j, sl, :, np.arange(CG) * 4 + b] = wdef[g]
                for r in range(2):
                    woffm[j, sl, :, r * 36 + b * 9:r * 36 + b * 9 + 9] = \
                        wofk[g, r].transpose(1, 0, 2, 3).reshape(CG, 9, 9).transpose(0, 2, 1)
                    boffb[j, r, np.arange(9) + b * 9, 0] = bofk[g, r]
            tbv[j, :, 0] = np.repeat(tbias[g], 4)        # row o*4+b -> t[o]
        maps.append(dict(
            xslab=xslab.astype(bfloat16), xres=xres.astype(bfloat16),
            wdefk=wdefk.reshape(NB, 128, 9 * 128).astype(bfloat16),
            woffm=woffm.reshape(NB, 128, 9 * 72).astype(bfloat16),
            boffb=boffb, tbias=tbv,
            eye=np.eye(128, dtype=bfloat16)))
    return maps


def _build_nc():
    nc = bacc.Bacc(None, target_bir_lowering=False)
    d_xs = nc.dram_tensor("xslab", [NB, 128, NGRID], BF16, kind="ExternalInput")
    d_xr = nc.dram_tensor("xres", [NB, 128, NOUT], BF16, kind="ExternalInput")
    d_wd = nc.dram_tensor("wdefk", [NB, 128, 9 * 128], BF16, kind="ExternalInput")
    d_wo = nc.dram_tensor("woffm", [NB, 128, 9 * 72], BF16, kind="ExternalInput")
    d_bo = nc.dram_tensor("boffb", [NB, 2, 36, 1], F32, kind="ExternalInput")
    d_tb = nc.dram_tensor("tbias", [NB, 128, 1], F32, kind="ExternalInput")
    d_eye = nc.dram_tensor("eye", [128, 128], BF16, kind="ExternalInput")
    d_out = nc.dram_tensor("out", [NB, 128, NOUT], BF16, kind="ExternalOutput")

    with tile.TileContext(nc) as tc:
        with (
            tc.tile_pool(name="xs", bufs=2) as p_xs,
            tc.tile_pool(name="wts", bufs=2) as p_w,
            tc.tile_pool(name="sm", bufs=2) as p_sm,
            tc.tile_pool(name="tents", bufs=1) as p_t,
            tc.tile_pool(name="um", bufs=1) as p_um,
            tc.tile_pool(name="urep", bufs=4) as p_ur,
            tc.tile_pool(name="fk", bufs=4) as p_f,
            tc.tile_pool(name="acc", bufs=2) as p_a,
            tc.tile_pool(name="tmp", bufs=8) as p_tmp,
            tc.tile_pool(name="oo", bufs=2) as p_o,
            tc.tile_pool(name="eye", bufs=1) as p_eye,
            tc.tile_pool(name="umd", bufs=2, space="DRAM") as p_umd,
            tc.tile_pool(name="psacc", bufs=1, space=bass.MemorySpace.PSUM) as p_pa,
            tc.tile_pool(name="psoff", bufs=1, space=bass.MemorySpace.PSUM) as p_po,
            tc.tile_pool(name="psf", bufs=2, space=bass.MemorySpace.PSUM) as p_pf,
        ):
            eye = p_eye.tile([128, 128], BF16)
            nc.sync.dma_start(eye[:], d_eye[:])
            umd_reads = {0: [], 1: []}
            umd_write = {0: None, 1: None}

            def phase_off(j):
                """IO + offset conv + tents + u products + umd write."""
                xs = p_xs.tile([128, NGRID], BF16, name="xs")
                nc.sync.dma_start(xs[:], d_xs[j])
                wdk = p_w.tile([128, 9 * 128], BF16, tag="wdk")
                nc.sync.dma_start(wdk[:], d_wd[j])
                wof = p_w.tile([128, 9 * 72], BF16, tag="wof")
                nc.sync.dma_start(wof[:], d_wo[j])
                bo_y = p_sm.tile([36, 1], F32, tag="bo_y")
                nc.sync.dma_start(bo_y[:], d_bo[j, 0])
                bo_x = p_sm.tile([36, 1], F32, tag="bo_x")
                nc.sync.dma_start(bo_x[:], d_bo[j, 1])
                tb = p_sm.tile([128, 1], F32, tag="tb")
                nc.sync.dma_start(tb[:], d_tb[j])

                xs_ap = xs[:]
                xs_pstep = xs_ap.ap[0][0]

                def xs_view(base, nrows, ncols=96):
                    return bass.AP(xs.tensor, xs_ap.offset + base,
                                   [[xs_pstep, 128], [GC, nrows], [1, ncols]])

                # ---- offset conv: two [36, 384] psum sets x 3 chunks ----
                offf = {}
                for r, bo_t in ((0, bo_y), (1, bo_x)):
                    of = p_t.tile([36, NOUT], F32, tag=f"off{r}")
                    pss = [p_po.tile([36, 384], F32, name=f"pss{r}_{c}", tag=f"pss{c}") for c in range(3)]
                    for p in range(9):
                        py, px = p // 3 - 1, p % 3 - 1
                        for ch in range(3):
                            rhs = xs_view((2 + py + ch * 4) * GC + 2 + px, 4)
                            nc.tensor.matmul(
                                pss[ch][:], wof[:, p * 72 + r * 36:p * 72 + r * 36 + 36],
                                rhs, start=(p == 0), stop=(p == 8))
                    for ch in range(3):
                        nc.scalar.activation(of[:, ch * 384:(ch + 1) * 384], pss[ch][:],
                                             mybir.ActivationFunctionType.Identity,
                                             bias=bo_t[:], scale=1.0)
                    offf[r] = of
                # ---- clamp + tents (per direction) ----
                uu = {}
                for r in (0, 1):
                    of = offf[r]
                    nc.vector.tensor_scalar(of[:], of[:], -CLAMP, CLAMP,
                                            ALU.max, ALU.min)
                    up = p_t.tile([36, NOUT], F32, tag=f"up{r}")
                    nc.vector.tensor_scalar(up[:], of[:], 0.0, None, ALU.max)
                    un = p_t.tile([36, NOUT], F32, tag=f"un{r}")
                    nc.vector.tensor_scalar(un[:], of[:], -1.0, 0.0, ALU.mult, ALU.max)
                    u0 = p_t.tile([36, NOUT], F32, tag=f"u0{r}")
                    nc.vector.tensor_tensor(u0[:], up[:], un[:], ALU.add)
                    nc.vector.tensor_scalar(u0[:], u0[:], -1.0, 1.0, ALU.mult, ALU.add)
                    uu[r] = {-1: un, 0: u0, 1: up}

                # ---- u-master [36, 9*NOUT] bf16, round-trip through DRAM ----
                umt = p_um.tile([36, 9 * NOUT], BF16)
                for dy in (-1, 0, 1):
                    for dx in (-1, 0, 1):
                        di = (dy + 1) * 3 + (dx + 1)
                        nc.vector.tensor_tensor(
                            umt[:, di * NOUT:(di + 1) * NOUT],
                            uu[0][dy][:], uu[1][dx][:], ALU.mult)
                umd = p_umd.tile([36, 9 * NOUT], BF16)
                slot = j % 2
                wr = nc.scalar.dma_start(umd[:], umt[:])
                for rd in umd_reads[slot]:
                    add_dep_helper(wr.ins, rd.ins, reason="umd WAR")
                umd_reads[slot] = []
                umd_write[slot] = wr
                xr = p_o.tile([128, NOUT], BF16, tag="xr", name="xr")
                nc.sync.dma_start(xr[:], d_xr[j])
                return dict(xs=xs, wdk=wdk, tb=tb, umd=umd, xr=xr, slot=slot,
                            urq=None, f0=None)

            def bcast_u(k, umd, slot):
                umd_ap = umd[:]
                umd_rowstep = umd_ap.ap[0][0]
                ur = p_ur.tile([128, 9 * NOUT], BF16, name="ur")
                ur_ap = ur[:]
                ur_pstep = ur_ap.ap[0][0]
                for b in range(4):
                    src = bass.AP(umd.tensor,
                                  umd_ap.offset + (b * 9 + k) * umd_rowstep,
                                  [[0, 32], [1, 9 * NOUT]])
                    dst = bass.AP(ur.tensor, ur_ap.offset + b * ur_pstep,
                                  [[4 * ur_pstep, 32], [1, 9 * NOUT]])
                    eng = nc.sync if b % 2 == 0 else nc.scalar
                    rd = eng.dma_start(dst, src)
                    add_dep_helper(rd.ins, umd_write[slot].ins, reason="umd RAW")
                    umd_reads[slot].append(rd)
                return ur

            def produce_f(st, k):
                xs, wdk = st["xs"], st["wdk"]
                fte = p_f.tile([128, NGRID], BF16, name="fte", tag="fte")
                for c4 in range(4):
                    psf = p_pf.tile([128, 400], F32, name="psf")
                    nc.tensor.matmul(psf[:], wdk[:, k * 128:(k + 1) * 128],
                                     xs[:, c4 * 400:(c4 + 1) * 400],
                                     start=True, stop=True)
                    nc.scalar.copy(fte[:, c4 * 400:(c4 + 1) * 400], psf[:])
                fto = p_f.tile([128, NGRID], BF16, name="fto", tag="fto")
                nc.scalar.copy(fto[:, 0:NGRID - 1], fte[:, 1:NGRID])
                return fte, fto

            def phase_main(j, st, nxt_st):
                xs, wdk, tb, umd, xr = st["xs"], st["wdk"], st["tb"], st["umd"], st["xr"]
                slot = st["slot"]

                # ---- main loop: mult on DVE, accumulate on PE into PSUM.
                # F production and the u-broadcast run one tap ahead so the
                # DVE never waits on the PE->ACT F pipeline. ----
                accs = [p_pa.tile([128, 384], F32, name=f"acc{c}", tag=f"acc{c}") for c in range(3)]


                npass = 0
                urq = st["urq"] or [bcast_u(0, umd, slot), bcast_u(1, umd, slot)]
                fnxt = st["f0"] or produce_f(st, 0)
                for k in range(9):
                    fte, fto = fnxt
                    ur = urq.pop(0)
                    if k < 8:
                        fnxt = produce_f(st, k + 1)
                    if k + 2 <= 8:
                        urq.append(bcast_u(k + 2, umd, slot))
                    if k == 7 and nxt_st is not None:
                        nxt_st["urq"] = [bcast_u(0, nxt_st["umd"], nxt_st["slot"]),
                                         bcast_u(1, nxt_st["umd"], nxt_st["slot"])]
                    if k == 8 and nxt_st is not None:
                        nxt_st["f0"] = produce_f(nxt_st, 0)
                    fte_pstep = fte[:].ap[0][0]
                    ftaps = {0: (fte, fte[:].offset), 1: (fto, fto[:].offset)}
                    for dy in (-1, 0, 1):
                        for dx in (-1, 0, 1):
                            di = (dy + 1) * 3 + (dx + 1)
                            base = (2 + KY[k] + dy) * GC + 2 + KX[k] + dx
                            ftile, foff = ftaps[base % 2]
                            ebase = base if base % 2 == 0 else base - 1
                            fview = bass.AP(
                                ftile.tensor, foff + ebase,
                                [[fte_pstep, 128], [GC, ROWS], [1, 96]])
                            usl = ur[:, di * NOUT:(di + 1) * NOUT]
                            tmp = p_tmp.tile([128, NOUT], BF16, name="tmp")
                            nc.vector.tensor_tensor(tmp[:], fview, usl, ALU.mult)
                            for c in range(3):
                                nc.tensor.matmul(
                                    accs[c][:], eye[:],
                                    tmp[:, c * 384:(c + 1) * 384],
                                    start=(npass == 0), stop=(npass == 80),
                                    skip_group_check=True)
                            npass += 1

                # ---- bias + residual + relu ----
                ot = p_o.tile([128, NOUT], BF16, tag="ot", name="ot")
                for c in range(3):
                    csl = slice(c * 384, (c + 1) * 384)
                    nc.vector.scalar_tensor_tensor(ot[:, csl], accs[c][:], tb[:],
                                                   xr[:, csl], ALU.add, ALU.add)
                nc.vector.tensor_scalar(ot[:], ot[:], 0.0, None, ALU.max)
                nc.sync.dma_start(d_out[j], ot[:])

            st = phase_off(0)
            for j in range(NB):
                nxt_st = phase_off(j + 1) if j + 1 < NB else None
                phase_main(j, st, nxt_st)
                st = nxt_st
    nc.compile()
    return nc


def kernel(x, w_off, b_off, w_def, gamma, beta, run_mean, run_var):
    global _nc_cache
    x = np.ascontiguousarray(np.asarray(x, np.float32))
    wofk, bofk, wdef, tbias = _prep_weights(
        np.asarray(w_off, np.float32), np.asarray(b_off, np.float32),
        np.asarray(w_def, np.float32), np.asarray(gamma, np.float32),
        np.asarray(beta, np.float32), np.asarray(run_mean, np.float32),
        np.asarray(run_var, np.float32))
    in_maps = _host_pack(x, wofk, bofk, wdef, tbias)
    if _nc_cache is None:
        _nc_cache = _build_nc()
    res = run_bass_kernel_spmd(_nc_cache, in_maps, core_ids=list(range(8)))
    out = np.zeros((B, G * CG, H, W), np.float32)
    for c in range(8):
        # rows are o-major: p = o*4+b
        o = np.asarray(res.results[c]["out"]).astype(np.float32).reshape(NB, CG, B, ROWS, W)
        for j, (g, oc) in enumerate(BLOCKS[c * NB:(c + 1) * NB]):
            out[:, g * CG:(g + 1) * CG, 12 * oc:12 * oc + 12] = \
                o[j].transpose(1, 0, 2, 3)
    return out
